# revision 1
# baseline (speedup 1.0000x reference)
"""GroupedQueryAttention Trainium2 kernel (8 NeuronCores).

Sharding: core c -> (batch b = c//4, kv-group g = c%4). Each core computes
the 4 heads of its kv-group for its batch, attention outputs (transposed,
[head*HD, L]) are AllGather-ed among the 4 cores of each batch, then every
core computes a disjoint 512-column slice of the output projection; the host
concatenates the 8 column-slices.

v2 structure (vs the f32r baseline; sim 373939 -> ~219000 ns, rel err 3.8e-3):
- bf16 operands throughout the attention/output path (error budget ~0.5%,
  gate is 2e-2). fp8e4 DoubleRow for the Q and K projections: Wq/Wk are
  scaled x64 on the host so their entries sit in e4m3's normal range, and
  rmsnorm absorbs the scale exactly (eps is scaled x64^2 to compensate).
  K(DR) and V(bf16) interleave into one PSUM accumulation group.
- AV matmul runs with the probabilities as the stationary operand and
  [v | ones] as the moving operand, so each 128-query accumulator picks up
  its softmax row-sums as a free 129th column: the separate ones-matmul
  (~34us PE) and the reciprocal-broadcast matmul of the baseline disappear,
  and normalization becomes a per-partition tensor_scalar.
- Two key-tiles' scores pair into one 2-bank psum tile with a single exp
  over both: ACT (the attention-phase pacer) drops from 98us to ~83us.
- Program order: prologue A(0..7), then per chunk c: B(c) heads with
  A(4c+8+h) and C(c-2) tiles interleaved BETWEEN heads (their DVE/ACT
  chains never queue up in one block ahead of B's exp/mask work), then the
  AllGather. In sim_mode the gather is stubbed per-head (broadcast the
  head's aT into its ag quarter + load back) so the tail only waits on the
  last head's own chain.
- DMA queues split by dependency chain: SP = x/trig/weights + AllGather
  chain, ACT = kv/out psum copies + out stores, Pool = attn stores (DMA
  waits hold their queue; engine-op waits don't).
- All DRAM inputs are host-packed so every DMA descriptor moves >=512B
  contiguous per partition (no small-descriptor penalty).
- PSUM plan (8 banks, one accumulation group per 2KB bank region):
  q-proj 1 (C tiles borrow it between A segments), transposes 1,
  scores 2x2 (kv-proj and tail C tiles borrow slices), attn-acc 2.
- Pitfalls encoded here: GPSIMD must not touch PSUM (walrus rejects);
  weight DMAs must be emitted before their first reader (dep tracking is
  emission-ordered); ag loads need an explicit add_dep_helper edge on the
  collective plus a chunk of slack.
"""

import numpy as np
import ml_dtypes

import concourse.bacc as bacc
import concourse.tile as tile
from concourse import mybir
from concourse.bass_utils import run_bass_kernel_spmd

F32 = mybir.dt.float32
BF16 = mybir.dt.bfloat16
FP8 = mybir.dt.float8e4
AF = mybir.ActivationFunctionType
ALU = mybir.AluOpType
DR = mybir.MatmulPerfMode.DoubleRow

B, L, D = 2, 2048, 2048
H, G, HD = 16, 4, 128
GS = H // G  # heads per kv group = 4
NCORES = 8
CHUNK = 512  # query-chunk (psum bank width in f32)
NLT = L // 128  # 16 row-tiles
NDK = D // 128  # 16 contraction-tiles
NCH = L // CHUNK  # 4 query chunks
EPS = 1e-6
WQ_SCALE = 64.0  # host-side Wq scale into fp8 range; rmsnorm absorbs it
SM_SCALE = 1.0 / float(HD * HD)

REPLICA_GROUPS = [[0, 1, 2, 3], [4, 5, 6, 7]]

_CACHE = {}
LAST_RESULT = None  # BassKernelResults of the most recent run (for test harness)


def _build_bass(sim_mode=False):
    # Bacc (not raw Bass): its compile() runs move_matmul_waits_to_ldweights
    # + generate_event_semaphores, required to satisfy the 1-wait-per-
    # instruction hardware constraint that walrus enforces.
    nc = bacc.Bacc("TRN2", target_bir_lowering=False, debug=False)

    # host-packed [p, tile, cols] layouts (partition-contiguous rows)
    xt8 = nc.declare_dram_parameter("xt8", [NLT, 128, NDK * 128], FP8, isOutput=False)
    xtb = nc.declare_dram_parameter("xtb", [NLT, 128, NDK * 128], BF16, isOutput=False)
    wq8 = nc.declare_dram_parameter("wq8", [128, NDK, GS * HD], FP8, isOutput=False)
    wk8 = nc.declare_dram_parameter("wk8", [128, NDK, HD], FP8, isOutput=False)
    wv = nc.declare_dram_parameter("wv", [128, NDK, HD], BF16, isOutput=False)
    wo = nc.declare_dram_parameter("wo", [128, H, CHUNK], BF16, isOutput=False)
    trig = nc.declare_dram_parameter("trig", [NLT, 128, 2 * GS * HD + 2 * HD], BF16,
                                     isOutput=False)
    maskd = nc.declare_dram_parameter("maskd", [CHUNK, CHUNK], BF16, isOutput=False)
    ident = nc.declare_dram_parameter("ident", [128, 128], BF16, isOutput=False)
    out = nc.declare_dram_parameter("out", [L, CHUNK], F32, isOutput=True)

    maskd_v = maskd[:].rearrange("(t p) n -> p t n", p=128)
    TRIGW = 2 * GS * HD + 2 * HD  # 1280

    from contextlib import ExitStack

    with tile.TileContext(nc) as tc, ExitStack() as stk:
        ent = stk.enter_context
        persist = ent(tc.tile_pool(name="persist", bufs=1))
        consts = ent(tc.tile_pool(name="consts", bufs=1))
        wts = ent(tc.tile_pool(name="wts", bufs=1))
        xin = ent(tc.tile_pool(name="xin", bufs=3))
        trigp = ent(tc.tile_pool(name="trigp", bufs=3))
        scrA = ent(tc.tile_pool(name="scrA", bufs=3))
        wTp = ent(tc.tile_pool(name="wTp", bufs=18))
        scrB = ent(tc.tile_pool(name="scrB", bufs=4))
        anp = ent(tc.tile_pool(name="anp", bufs=3))
        aTp = ent(tc.tile_pool(name="aTp", bufs=2))
        aginpool = ent(tc.tile_pool(name="agin", bufs=2))
        outpool = ent(tc.tile_pool(name="outsb", bufs=2))
        ps_q = ent(tc.tile_pool(name="ps_q", bufs=1, space="PSUM"))
        ps_tr = ent(tc.tile_pool(name="ps_tr", bufs=1, space="PSUM"))
        ps_s = ent(tc.tile_pool(name="ps_s", bufs=2, space="PSUM"))
        ps_acc = ent(tc.tile_pool(name="ps_acc", bufs=2, space="PSUM"))
        ccpool = ent(tc.tile_pool(name="cc", bufs=2, space="DRAM"))
        ccagpool = ent(tc.tile_pool(name="ccag", bufs=4, space="DRAM"))
        if True:
            # persistent SBUF
            qT_sb = persist.tile([128, GS, L], BF16)  # [hd, head, l]
            kT_sb = persist.tile([128, L], BF16)  # [hd, l]
            v_sb = persist.tile([128, NLT, 130], BF16)  # [l, lt, hd|1|pad]
            epsq_sb = consts.tile([128, 1], F32)
            epsk_sb = consts.tile([128, 1], F32)
            maskd_sb = consts.tile([128, NCH, CHUNK], BF16)
            ident_sb = consts.tile([128, 128], BF16)
            nc.sync.dma_start(ident_sb[:], ident[:])
            nc.gpsimd.memset(epsq_sb[:], EPS * WQ_SCALE * WQ_SCALE)
            nc.gpsimd.memset(epsk_sb[:], EPS * WQ_SCALE * WQ_SCALE)
            nc.gpsimd.memset(v_sb[:, :, 128:130], 1.0)

            # weight/ag bulk loads go on the Pool queue (cheap DMA dispatch,
            # no contention with the exp-saturated ACT queue); wo is deferred
            # until the c=1 block since phase C first needs it ~150us in.
            wk8_sb = wts.tile([128, NDK, HD], FP8)
            wv_sb = wts.tile([128, NDK, HD], BF16)
            wq8_sb = wts.tile([128, NDK, GS * HD], FP8)
            wo_sb = wts.tile([128, H, CHUNK], BF16)
            # weight loads are interleaved into the first A-segment's
            # emission below so the first x tiles transfer first

            # ---------------- phase emitters --------------------------------
            def emit_A_dma(lt):
                x8 = xin.tile([128, NDK, 128], FP8, tag="x8")
                xb = xin.tile([128, NDK, 128], BF16, tag="xb")
                nc.sync.dma_start(x8[:], xt8[lt])
                nc.sync.dma_start(xb[:], xtb[lt])
                tg = trigp.tile([128, TRIGW], BF16, tag="tg")
                nc.sync.dma_start(tg[:], trig[lt])
                return x8, xb, tg

            def emit_A(lt, pre=None):
                """projections + rmsnorm + rope + transposes for one 128-row
                token tile."""
                ls = slice(lt * 128, (lt + 1) * 128)
                x8, xb, tg = pre if pre is not None else emit_A_dma(lt)
                cq_t = tg[:, 0:GS * HD]
                sq_t = tg[:, GS * HD:2 * GS * HD]
                ck_t = tg[:, 2 * GS * HD:2 * GS * HD + HD]
                sk_t = tg[:, 2 * GS * HD + HD:TRIGW]

                # K (fp8 DoubleRow) and V (bf16) projections share one PSUM
                # bank as a single interleaved accumulation group; the bank
                # is borrowed from the scores pool (idle during A segments)
                kv_tile = ps_s.tile([128, 2, CHUNK], F32, tag="s")
                kv_ps = kv_tile[:][:, 0, 0:2 * HD]
                for t in range(NDK // 2):
                    nc.tensor.matmul(
                        kv_ps[:, 0:HD], x8[:, 2 * t:2 * t + 2, :],
                        wk8_sb[:, 2 * t:2 * t + 2, :],
                        start=(t == 0), stop=False, perf_mode=DR,
                    )
                    nc.tensor.matmul(
                        kv_ps[:, HD:2 * HD], xb[:, 2 * t, :],
                        wv_sb[:, 2 * t, :], start=False, stop=False,
                    )
                    nc.tensor.matmul(
                        kv_ps[:, HD:2 * HD], xb[:, 2 * t + 1, :],
                        wv_sb[:, 2 * t + 1, :], start=False,
                        stop=(t == NDK // 2 - 1),
                    )
                q_ps = ps_q.tile([128, GS * HD], F32, tag="q")
                for t in range(NDK // 2):
                    nc.tensor.matmul(
                        q_ps[:], x8[:, 2 * t:2 * t + 2, :],
                        wq8_sb[:, 2 * t:2 * t + 2, :],
                        start=(t == 0), stop=(t == NDK // 2 - 1),
                        perf_mode=DR,
                    )

                # PSUM->SBUF copies on Pool: frees DVE, which is the phase-A
                # bottleneck engine
                qsb = scrA.tile([128, GS * HD], BF16, tag="qsb")
                kvsb = scrA.tile([128, 2 * HD], BF16, tag="kvsb")
                nc.vector.tensor_copy(qsb[:], q_ps[:])
                nc.scalar.copy(kvsb[:], kv_ps)
                nc.vector.tensor_copy(v_sb[:, lt, 0:HD], kvsb[:, HD:2 * HD])

                # rmsnorm stats (free-dim reduce per head)
                sq_full = scrA.tile([128, GS * HD], BF16, tag="sqf")
                sums = scrA.tile([128, 8], F32, tag="sums")
                rms = scrA.tile([128, 8], F32, tag="rms")
                recip = scrA.tile([128, 8], F32, tag="recip")
                nc.vector.tensor_mul(sq_full[:], qsb[:], qsb[:])
                nc.vector.reduce_sum(
                    sums[:, 0:GS],
                    sq_full[:].rearrange("p (h d) -> p h d", h=GS),
                    axis=mybir.AxisListType.X,
                )
                sq_k = scrA.tile([128, HD], BF16, tag="sqk")
                nc.vector.tensor_mul(sq_k[:], kvsb[:, 0:HD], kvsb[:, 0:HD])
                nc.vector.reduce_sum(
                    sums[:, GS:GS + 1], sq_k[:], axis=mybir.AxisListType.X
                )
                nc.scalar.activation(
                    rms[:, 0:GS], sums[:, 0:GS], AF.Sqrt,
                    scale=1.0 / HD, bias=epsq_sb[:],
                )
                nc.scalar.activation(
                    rms[:, GS:GS + 1], sums[:, GS:GS + 1], AF.Sqrt,
                    scale=1.0 / HD, bias=epsk_sb[:],
                )
                nc.vector.reciprocal(recip[:, 0:GS + 1], rms[:, 0:GS + 1])

                # normalize (q_scale/k_scale are baked into the trig tables)
                qn = scrA.tile([128, GS * HD], BF16, tag="qn")
                for h in range(GS):
                    hs = slice(h * HD, (h + 1) * HD)
                    nc.vector.tensor_scalar_mul(
                        qn[:, hs], qsb[:, hs], recip[:, h:h + 1]
                    )
                kn = scrA.tile([128, HD], BF16, tag="kn")
                nc.vector.tensor_scalar_mul(
                    kn[:], kvsb[:, 0:HD], recip[:, GS:GS + 1]
                )

                # rope: qr = qn*cos' + swap_halves(qn)*sin' (sign in sin')
                hh = HD // 2
                t1q = scrA.tile([128, GS * HD], BF16, tag="t1q")
                t2q = scrA.tile([128, GS * HD], BF16, tag="t2q")
                qr = scrA.tile([128, GS * HD], BF16, tag="qr")
                nc.vector.tensor_mul(t1q[:], qn[:], cq_t)
                qn3 = qn[:].rearrange("p (h d) -> p h d", h=GS)
                t23 = t2q[:].rearrange("p (h d) -> p h d", h=GS)
                sq3 = sq_t.rearrange("p (h d) -> p h d", h=GS)
                nc.vector.tensor_mul(
                    t23[:, :, 0:hh], qn3[:, :, hh:HD], sq3[:, :, 0:hh]
                )
                nc.vector.tensor_mul(
                    t23[:, :, hh:HD], qn3[:, :, 0:hh], sq3[:, :, hh:HD]
                )
                nc.vector.tensor_add(qr[:], t1q[:], t2q[:])

                t1k = scrA.tile([128, HD], BF16, tag="t1k")
                t2k = scrA.tile([128, HD], BF16, tag="t2k")
                kr = scrA.tile([128, HD], BF16, tag="kr")
                nc.vector.tensor_mul(t1k[:], kn[:], ck_t)
                nc.vector.tensor_mul(t2k[:, 0:hh], kn[:, hh:HD], sk_t[:, 0:hh])
                nc.vector.tensor_mul(t2k[:, hh:HD], kn[:, 0:hh], sk_t[:, hh:HD])
                nc.vector.tensor_add(kr[:], t1k[:], t2k[:])

                # transpose q/k to [hd, l] (v stays natural)
                tr = ps_tr.tile([128, 5, 128], BF16, tag="tr")
                for h in range(GS):
                    hs = slice(h * HD, (h + 1) * HD)
                    nc.tensor.transpose(tr[:, h, :], qr[:, hs], ident_sb[:])
                nc.tensor.transpose(tr[:, GS, :], kr[:], ident_sb[:])
                nc.vector.tensor_copy(qT_sb[:, :, ls], tr[:, 0:GS, :])
                nc.vector.tensor_copy(kT_sb[:, ls], tr[:, GS, :])

            def emit_B_head(c, h, attn_my, ag_out, ag_sb):
                """attention for one (query-chunk, head)."""
                cs = slice(c * CHUNK, (c + 1) * CHUNK)
                njt = 4 * (c + 1)  # causal: key tiles 0 .. 4c+3
                wts_l = []
                for pr in range(njt // 2):
                    # two key-tiles' scores into one 2-bank psum tile, one
                    # exp over both (halves ACT time, the B-phase pacer)
                    s_ps = ps_s.tile([128, 2, CHUNK], F32, tag="s")
                    for j in range(2):
                        jt = 2 * pr + j
                        js = slice(jt * 128, (jt + 1) * 128)
                        nc.tensor.matmul(
                            s_ps[:, j, :], kT_sb[:, js], qT_sb[:, h, cs]
                        )
                    wT = wTp.tile([128, 2, CHUNK], BF16, tag="w")
                    nc.scalar.activation(wT[:], s_ps[:], AF.Exp, scale=SM_SCALE)
                    for j in range(2):
                        jt = 2 * pr + j
                        jd = jt - 4 * c
                        if jd >= 0:  # diagonal band: apply causal mask
                            nc.vector.tensor_mul(
                                wT[:, j, :], wT[:, j, :], maskd_sb[:, jd, :]
                            )
                    wts_l.append(wT)
                a_nf = anp.tile([128, 4, 128], BF16, tag="anf")
                for qc in range(4):
                    qs = slice(qc * 128, (qc + 1) * 128)
                    acc = ps_acc.tile([128, 129], F32, tag="acc")
                    for jt in range(njt):
                        nc.tensor.matmul(
                            acc[:], wts_l[jt // 2][:, jt % 2, qs],
                            v_sb[:, jt, 0:129],
                            start=(jt == 0), stop=(jt == njt - 1),
                        )
                    rec = scrB.tile([128, 1], F32, tag="rec")
                    nc.vector.reciprocal(rec[:], acc[:, 128:129])
                    nc.vector.tensor_scalar_mul(
                        a_nf[:, qc, :], acc[:, 0:128], rec[:]
                    )
                tr = ps_tr.tile([128, 5, 128], BF16, tag="tr")
                for qc in range(4):
                    nc.tensor.transpose(
                        tr[:, qc, :], a_nf[:][:, qc, :], ident_sb[:]
                    )
                aT = aTp.tile([128, 4, 128], BF16, tag="aT")
                nc.vector.tensor_copy(aT[:], tr[:, 0:4, :])
                nc.gpsimd.dma_start(
                    attn_my[h * HD:(h + 1) * HD, :],
                    aT[:].rearrange("p q d -> p (q d)"),
                )
                if sim_mode:
                    # per-head AllGather stand-in: broadcast this head's aT
                    # into its ag quarter and load it back, so the post-B
                    # tail only waits on the last head's own chain
                    ag_q = ag_out[h * GS * HD:(h + 1) * GS * HD, :]
                    nc.sync.dma_start(
                        ag_q.rearrange("(r p) n -> p r n", p=128),
                        aT[:].rearrange("p q d -> p (q d)")
                        .rearrange("p (o n) -> p o n", o=1)
                        .to_broadcast([128, 4, CHUNK]),
                    )
                    ag_v = ag_out[:].rearrange("(t p) n -> p t n", p=128)
                    nc.sync.dma_start(
                        ag_sb[:, 4 * h:4 * h + 4, :],
                        ag_v[:, 4 * h:4 * h + 4, :],
                    )

            ag_sbs = {}
            ag_ccs = {}

            def emit_AG(c, attn_my, ag_out, ag_sb):
                """Real mode: AllGather collective; the SBUF loads are
                emitted a chunk later (emit_real_loads) for slack against
                the collective's completion signalling. (In sim_mode the
                per-head stubs in emit_B_head already populated
                ag_out/ag_sb.)"""
                if sim_mode:
                    return ag_out
                cc = nc.gpsimd.collective_compute(
                    "AllGather",
                    ALU.bypass,
                    ins=[attn_my.opt()],
                    outs=[ag_out.opt()],
                    replica_groups=REPLICA_GROUPS,
                )
                ag_ccs[c] = cc
                return ag_out

            def emit_real_loads(c):
                ag_v = ag_outs[c][:].rearrange("(t p) n -> p t n", p=128)
                ag_sb = ag_sbs[c]
                for g2 in range(4):
                    ld = nc.sync.dma_start(
                        ag_sb[:, 4 * g2:4 * g2 + 4, :],
                        ag_v[:, 4 * g2:4 * g2 + 4, :],
                    )
                    # the DRAM-side dependency on the collective's output is
                    # not reliably tracked through .opt(); make it explicit
                    tile.add_dep_helper(
                        ld.ins, ag_ccs[c].ins, reason="ag load after collective"
                    )

            def emit_C_tile(c, it, alt_bank=False):
                ag_sb = ag_sbs[c]
                its = slice(it * 128, (it + 1) * 128)
                # no dedicated bank: borrow the q-projection bank (idle
                # between A segments); in the tail alternate with a scores
                # bank slice to double-buffer
                if alt_bank:
                    o_tile = ps_s.tile([128, 2, CHUNK], F32, tag="s")
                    o_ps = o_tile[:][:, 0, :]
                else:
                    o_tile = ps_q.tile([128, GS * HD], F32, tag="q")
                    o_ps = o_tile[:]
                for t in range(H):
                    nc.tensor.matmul(
                        o_ps, ag_sb[:, t, its], wo_sb[:, t, :],
                        start=(t == 0), stop=(t == H - 1),
                    )
                o_sb = outpool.tile([128, CHUNK], F32, tag="o_sb")
                nc.scalar.copy(o_sb[:], o_ps)
                # ACT queue: a store holding the Pool queue would block the
                # next tile's o_sb copy dispatch and serialize the tail
                nc.scalar.dma_start(
                    out[c * CHUNK + it * 128:c * CHUNK + (it + 1) * 128, :],
                    o_sb[:],
                )

            # ---------------- staircase schedule ----------------------------
            # staircase, two chunks of A-prefetch, with the A tiles spread
            # between B heads so their DVE/ACT chains never queue up in one
            # block ahead of B's exp/mask work
            # NB: weight loads MUST be emitted before their first reader
            # (emit_A(0)) — the dependency tracker only links reads to
            # prior writes in emission order
            ag_outs = {}
            pre0 = emit_A_dma(0)
            nc.sync.dma_start(wk8_sb[:], wk8[:])
            nc.sync.dma_start(wv_sb[:], wv[:])
            nc.sync.dma_start(wq8_sb[:, 0:NDK // 2, :], wq8[:, 0:NDK // 2, :])
            nc.sync.dma_start(wq8_sb[:, NDK // 2:, :], wq8[:, NDK // 2:, :])
            emit_A(0, pre=pre0)
            nc.sync.dma_start(maskd_sb[:], maskd_v)
            for lt in range(1, 8):
                emit_A(lt)
            for c in range(NCH):
                if c == 1:
                    # hold the wo load off the startup DMA burst (the tile
                    # scheduler would otherwise hoist this dependency-free
                    # 2MB transfer in front of the first x tiles)
                    with tc.tile_wait_until(0.06):
                        nc.gpsimd.dma_start(wo_sb[:], wo[:])
                attn_my = ccpool.tile([GS * HD, CHUNK], BF16, tag="attn_my")
                ag_out = ccagpool.tile([H * HD, CHUNK], BF16, tag="ag_out")
                ag_sb = aginpool.tile([128, H, CHUNK], BF16, tag="ag")
                ag_sbs[c] = ag_sb
                for h in range(GS):
                    emit_B_head(c, h, attn_my, ag_out, ag_sb)
                    lt = 4 * c + 8 + h
                    if lt < NLT:
                        emit_A(lt)
                    if c >= 2:
                        emit_C_tile(c - 2, h)
                ag_outs[c] = emit_AG(c, attn_my, ag_out, ag_sb)
                if not sim_mode and c >= 1:
                    emit_real_loads(c - 1)
            for it in range(4):
                emit_C_tile(2, it, alt_bank=(it % 2 == 1))
            if not sim_mode:
                emit_real_loads(3)
            for it in range(4):
                emit_C_tile(3, it, alt_bank=(it % 2 == 1))
    nc.compile()
    return nc


def _get_nc():
    if "nc" not in _CACHE:
        _CACHE["nc"] = _build_bass()
    return _CACHE["nc"]


def kernel(x, Wq, Wk, Wv, Wo, q_scale, k_scale, cos, sin, mask):
    global LAST_RESULT
    nc = _get_nc()

    f32 = np.float32
    bf16 = ml_dtypes.bfloat16
    fp8 = ml_dtypes.float8_e4m3fn
    x = np.asarray(x, f32)
    cos = np.asarray(cos, f32)
    sin = np.asarray(sin, f32)
    q_scale = np.asarray(q_scale, f32)
    k_scale = np.asarray(k_scale, f32)

    sgn = np.concatenate([-np.ones(HD // 2, f32), np.ones(HD // 2, f32)])
    qs_swap = np.concatenate([q_scale[HD // 2:], q_scale[:HD // 2]])
    ks_swap = np.concatenate([k_scale[HD // 2:], k_scale[:HD // 2]])
    cosq = np.tile(cos * q_scale[None, :], (1, GS))  # [L, GS*HD]
    sinq = np.tile(sin * (sgn * qs_swap)[None, :], (1, GS))
    cosk = cos * k_scale[None, :]  # [L, HD]
    sink = sin * (sgn * ks_swap)[None, :]
    trig_full = np.concatenate([cosq, sinq, cosk, sink], axis=1)  # [L, 1280]
    trig_t = np.ascontiguousarray(
        trig_full.reshape(NLT, 128, -1).astype(bf16))  # [lt, p, 1280]

    # diagonal-band mask, key-major: 1.0 where key j' may attend query i'
    maskd = np.ascontiguousarray((~mask[:CHUNK, :CHUNK]).T.astype(bf16))

    # x tiled [lt, p, dk, l]: partition p = d-row within dk-tile
    xt8s, xtbs = [], []
    for b in range(B):
        xr = np.asarray(x[b], f32).reshape(NLT, 128, NDK, 128)  # [lt, l, dk, p]
        xr = np.ascontiguousarray(xr.transpose(0, 3, 2, 1))  # [lt, p, dk, l]
        xt8s.append(xr.astype(fp8).reshape(NLT, 128, NDK * 128))
        xtbs.append(xr.astype(bf16).reshape(NLT, 128, NDK * 128))

    def pack_pdn(w):  # [D, N] -> [p, dk, N]
        n = w.shape[1]
        return np.ascontiguousarray(
            w.reshape(NDK, 128, n).transpose(1, 0, 2))

    in_maps = []
    for core in range(NCORES):
        b, g = divmod(core, G)
        hs = slice(g * GS * HD, (g + 1) * GS * HD)
        gs = slice(g * HD, (g + 1) * HD)
        in_maps.append({
            "xt8": xt8s[b],
            "xtb": xtbs[b],
            "wq8": pack_pdn((np.asarray(Wq[:, hs], f32) * WQ_SCALE)).astype(fp8),
            "wk8": pack_pdn((np.asarray(Wk[:, gs], f32) * WQ_SCALE)).astype(fp8),
            "wv": pack_pdn(np.asarray(Wv[:, gs], f32)).astype(bf16),
            "wo": pack_pdn(np.asarray(Wo[:, hs], f32)).astype(bf16),
            "trig": trig_t,
            "maskd": maskd, "ident": np.eye(128, dtype=bf16),
        })

    res = run_bass_kernel_spmd(nc, in_maps, list(range(NCORES)))
    LAST_RESULT = res

    out = np.empty((B, L, D), f32)
    for core in range(NCORES):
        b, g = divmod(core, G)
        out[b, :, g * CHUNK:(g + 1) * CHUNK] = res.results[core]["out"]
    return out



# revision 21
# speedup vs baseline: 1.1948x; 1.1948x over previous
"""GroupedQueryAttention Trainium2 kernel (8 NeuronCores), v3 "linearized".

Sharding: core c -> (batch b = c//4, kv-group g = c%4). Each core computes
the 4 heads of its kv-group for its batch; per query-chunk the cores of a
batch AllGather (1) fp8 delta-attention outputs and (2) bf16 causal-mean
rows, then every core computes a disjoint 512-column slice of the output
projection; the host concatenates the 8 column-slices.

v3 structure (vs the v2 exp baseline; sim 212403 -> target ~140k):
- The reference divides scores by HD^2 = 16384, so post-rmsnorm logits are
  |s| <= 0.004 and exp(s) = 1+s to 6e-7 relative (validated vs reference).
  Softmax is LINEARIZED: w = (1+s)*mask. This splits attention exactly into
    acc = CSv (causal prefix-sum of v; the "1" part)
        + (s*mask) @ v          (the small part)
  and the small part off the 512-wide block diagonal collapses by
  associativity: s@v = q @ Z where Z = K^T [V|1] is a per-key-tile-prefix
  [128,129] matrix. Off-diagonal scores+AV (41us PE, ~450 matmuls in v2)
  become ~60 matmuls. exp disappears -> ACT runs only Copy/Sqrt, so the
  25 LoadActFuncSet table thrashes (32us) also disappear.
- Diag band (4 key tiles per chunk): scores psum is pre-scaled S_D*SM
  (baked into k's rope trig host-side), ACT pure-casts it to fp8 d8,
  Pool applies the 0/1 mask, and AV runs fp8 DoubleRow with v8 = fp8(v)
  (two key tiles per matmul). d8/v8 quantization only touches the small
  part (|d-part| ~ 7e-4 of acc) -> error ~3e-5.
- a = abar + delta_a with abar = CSfull/n (head-independent!) and
  delta_a = acc_d/(S_D*rs) taken STRAIGHT from the d-psum (no subtract).
  rs = n + accd_col/S_D. Dropping the per-head (n/rs) factor on abar is a
  2e-5 relative error (rs/n - 1 ~ s-mean ~ 2e-5).
- Out-proj: out = sum_h delta_a_h @ Wo_h  (fp8 DoubleRow, 8 mm/tile)
            + sum_g (CSfull_g/n) @ (sum_{h in g} Wo_h * S_A*64)  (4 mm/tile)
  so the AllGather carries fp8 delta (512x512) + bf16 CSnT (128x512) per
  chunk: 384KB vs 2MB.
- CSfullT [hd, l] built per tile as v_nat @ M_i (static bf16 tri/n
  matrices) + S-row inject (running column-sums of v, [1,128] stationary).
- V is projected TRANSPOSED chunk-wide (x repacked [c, d, dk, 512]):
  16 dk-matmuls per 256-wide half instead of 16 matmuls per 128-row tile
  (64+16tr vs 256 matmuls), then transposed back for v natural + v8.
- Why matmul COUNT matters: PE sequencer is the baseline bottleneck
  (~110ns dispatch per ldweights+matmult pair, 1712 pairs = 188us).
  v3 has ~980 matmuls. Matmul engine cost = out_free_size * 0.42ns
  (*0.5 fp8 DR), independent of contraction size.
- Scale bookkeeping: k-trig *= S_D*SM -> scores psum, Z, d8 all S_D-scaled;
  delta cast multiplies S_A/(S_D*rs) (recip fed accd_col/S_A + n*S_D/S_A);
  wo8 = 64*Wo fp8, wosum = sum Wo_h * S_A*64 bf16; out copy scale
  1/(S_A*64). rmsnorm absorbs WQ_SCALE for q (x8/wq8 fp8) as in v2.
- Pitfalls encoded: GPSIMD(Pool) must not touch PSUM; weight DMAs before
  first reader; ag loads need add_dep_helper on the collective; DMA queues
  split SP (x/trig/weights + collectives), ACT (psum copies + out stores),
  Pool (attn/csn stores); stationaries with K=1 ([1,128] rows) are legal.
"""

import numpy as np
import ml_dtypes

import concourse.bacc as bacc
import concourse.tile as tile
from concourse import mybir
from concourse.bass_utils import run_bass_kernel_spmd

F32 = mybir.dt.float32
BF16 = mybir.dt.bfloat16
FP8 = mybir.dt.float8e4
AF = mybir.ActivationFunctionType
ALU = mybir.AluOpType
DR = mybir.MatmulPerfMode.DoubleRow

B, L, D = 2, 2048, 2048
H, G, HD = 16, 4, 128
GS = H // G  # heads per kv group = 4
NCORES = 8
CHUNK = 512
NLT = L // 128  # 16 row-tiles
NDK = D // 128  # 16 contraction-tiles
NCH = L // CHUNK  # 4 query chunks
EPS = 1e-6
WQ_SCALE = 64.0  # host-side Wq/Wo scale into fp8 range
SM_SCALE = 1.0 / float(HD * HD)
S_D = 1024.0  # d8 (scaled scores) fp8 scale; baked with SM into k trig
S_A = 8192.0  # delta_a fp8 scale
OUT_DESCALE = 1.0 / (S_A * WQ_SCALE)

REPLICA_GROUPS = [[0, 1, 2, 3], [4, 5, 6, 7]]

_CACHE = {}
LAST_RESULT = None  # BassKernelResults of the most recent run (for test harness)


def _build_bass(sim_mode=False):
    nc = bacc.Bacc("TRN2", target_bir_lowering=False, debug=False)

    # host-packed layouts (partition-contiguous rows)
    xt8 = nc.declare_dram_parameter("xt8", [NLT, 128, NDK * 128], FP8, isOutput=False)
    xtv = nc.declare_dram_parameter("xtv", [NCH, 128, NDK * 512], BF16, isOutput=False)
    wq8 = nc.declare_dram_parameter("wq8", [128, NDK, GS * HD], FP8, isOutput=False)
    wk8 = nc.declare_dram_parameter("wk8", [128, NDK, HD], FP8, isOutput=False)
    wv = nc.declare_dram_parameter("wv", [128, NDK, HD], BF16, isOutput=False)
    wo8 = nc.declare_dram_parameter("wo8", [128, H, CHUNK], FP8, isOutput=False)
    wosum = nc.declare_dram_parameter("wosum", [128, G, CHUNK], BF16, isOutput=False)
    trig = nc.declare_dram_parameter("trig", [NLT, 128, 2 * GS * HD + 2 * HD], BF16,
                                     isOutput=False)
    maskd8 = nc.declare_dram_parameter("maskd8", [CHUNK, CHUNK], FP8, isOutput=False)
    mi = nc.declare_dram_parameter("mi", [128, NLT, 128], BF16, isOutput=False)
    nrow = nc.declare_dram_parameter("nrow", [1, NLT, 128], BF16, isOutput=False)
    nvec = nc.declare_dram_parameter("nvec", [128, NLT], F32, isOutput=False)
    ident = nc.declare_dram_parameter("ident", [128, 128], BF16, isOutput=False)
    out = nc.declare_dram_parameter("out", [L, CHUNK], F32, isOutput=True)

    maskd_v = maskd8[:].rearrange("(t p) n -> p t n", p=128)
    TRIGW = 2 * GS * HD + 2 * HD  # 1280

    from contextlib import ExitStack

    with tile.TileContext(nc) as tc, ExitStack() as stk:
        ent = stk.enter_context
        persist = ent(tc.tile_pool(name="persist", bufs=1))
        consts = ent(tc.tile_pool(name="consts", bufs=1))
        wts = ent(tc.tile_pool(name="wts", bufs=1))
        xin = ent(tc.tile_pool(name="xin", bufs=3))
        xvin = ent(tc.tile_pool(name="xvin", bufs=2))
        trigp = ent(tc.tile_pool(name="trigp", bufs=3))
        scrA = ent(tc.tile_pool(name="scrA", bufs=3))
        wTp = ent(tc.tile_pool(name="wTp", bufs=4))
        scrB = ent(tc.tile_pool(name="scrB", bufs=4))
        anp = ent(tc.tile_pool(name="anp", bufs=3))
        aTp = ent(tc.tile_pool(name="aTp", bufs=2))
        vtp = ent(tc.tile_pool(name="vtp", bufs=2))
        aginpool = ent(tc.tile_pool(name="agin", bufs=2))
        csinpool = ent(tc.tile_pool(name="csin", bufs=2))
        outpool = ent(tc.tile_pool(name="outsb", bufs=2))
        ps_q = ent(tc.tile_pool(name="ps_q", bufs=1, space="PSUM"))
        ps_tr = ent(tc.tile_pool(name="ps_tr", bufs=1, space="PSUM"))
        ps_s = ent(tc.tile_pool(name="ps_s", bufs=2, space="PSUM"))
        ps_acc = ent(tc.tile_pool(name="ps_acc", bufs=2, space="PSUM"))
        ccpool = ent(tc.tile_pool(name="cc", bufs=2, space="DRAM"))
        cspool = ent(tc.tile_pool(name="cs", bufs=2, space="DRAM"))
        ccagpool = ent(tc.tile_pool(name="ccag", bufs=4, space="DRAM"))
        csagpool = ent(tc.tile_pool(name="csag", bufs=4, space="DRAM"))
        if True:
            # persistent SBUF
            qT_sb = persist.tile([128, GS, L], BF16)  # [hd, head, l]
            kT_sb = persist.tile([128, L], BF16)  # [hd, l]
            krblk_sb = persist.tile([128, 4, HD], BF16)  # k natural, per block
            v_sb = persist.tile([128, NLT, 130], BF16)  # [l, lt, hd|1|pad]
            v8_sb = persist.tile([128, NLT, 130], FP8)
            csfT_sb = persist.tile([128, NLT, 128], BF16)  # [hd, lt, l] = CSfull^T/n
            srow_f = persist.tile([1, NLT + 1, 128], F32)  # running col-sums of v
            srow_b = persist.tile([1, NLT, 128], BF16)
            z_sb = persist.tile([128, NCH, 132], BF16)  # Z_c prefix [hd|sum_k]
            epsq_sb = consts.tile([128, 1], F32)
            epsk_sb = consts.tile([128, 1], F32)
            maskd_sb = consts.tile([128, NCH, CHUNK], FP8)
            mi_sb = consts.tile([128, NLT, 128], BF16)
            nrow_sb = consts.tile([1, NLT, 128], BF16)
            nvec_sb = consts.tile([128, NLT], F32)
            ident_sb = consts.tile([128, 128], BF16)
            ones_col = consts.tile([128, 1], BF16)
            nc.sync.dma_start(ident_sb[:], ident[:])
            nc.gpsimd.memset(epsq_sb[:], EPS * WQ_SCALE * WQ_SCALE)
            nc.gpsimd.memset(epsk_sb[:], EPS * WQ_SCALE * WQ_SCALE)
            nc.gpsimd.memset(v_sb[:, :, 128:130], 1.0)
            nc.gpsimd.memset(v8_sb[:, :, 128:130], 1.0)
            nc.gpsimd.memset(ones_col[:], 1.0)
            nc.gpsimd.memset(srow_f[:, 0, :], 0.0)

            wk8_sb = wts.tile([128, NDK, HD], FP8)
            wv_sb = wts.tile([128, NDK, HD], BF16)
            wq8_sb = wts.tile([128, NDK, GS * HD], FP8)
            wo8_sb = wts.tile([128, H, CHUNK], FP8)
            wosum_sb = wts.tile([128, G, CHUNK], BF16)

            # ---------------- phase emitters --------------------------------
            def emit_VT(c):
                """V projection for one 512-query chunk, transposed output
                [hd, l]; then transpose back to v natural + fp8 copy."""
                xv = xvin.tile([128, NDK, 512], BF16, tag="xv")
                nc.sync.dma_start(xv[:], xtv[c])
                for half in range(2):
                    hs = slice(half * 256, (half + 1) * 256)
                    vt_tile = ps_q.tile([128, GS * HD], F32, tag="q")
                    vt_ps = vt_tile[:, 0:256]
                    for t in range(NDK):
                        nc.tensor.matmul(
                            vt_ps, wv_sb[:, t, :], xv[:, t, hs],
                            start=(t == 0), stop=(t == NDK - 1),
                        )
                    vtsb = vtp.tile([128, 256], BF16, tag="vtsb")
                    nc.scalar.copy(vtsb[:], vt_ps)
                    for j in range(2):
                        lt = 4 * c + 2 * half + j
                        tr = ps_tr.tile([128, 5, 128], BF16, tag="tr")
                        nc.tensor.transpose(
                            tr[:, 0, :], vtsb[:, j * 128:(j + 1) * 128],
                            ident_sb[:],
                        )
                        nc.vector.tensor_copy(v_sb[:, lt, 0:HD], tr[:, 0, :])
                        nc.scalar.copy(v8_sb[:, lt, 0:HD], tr[:, 0, :])

            def emit_A_dma(lt):
                x8 = xin.tile([128, NDK, 128], FP8, tag="x8")
                nc.sync.dma_start(x8[:], xt8[lt])
                tg = trigp.tile([128, TRIGW], BF16, tag="tg")
                nc.sync.dma_start(tg[:], trig[lt])
                return x8, tg

            def emit_A(lt, pre=None):
                """q/k projections + rmsnorm + rope + transposes + Z/CS/S
                bookkeeping for one 128-row token tile. Needs v_sb[lt]
                (from emit_VT) already emitted."""
                ls = slice(lt * 128, (lt + 1) * 128)
                x8, tg = pre if pre is not None else emit_A_dma(lt)
                cq_t = tg[:, 0:GS * HD]
                sq_t = tg[:, GS * HD:2 * GS * HD]
                ck_t = tg[:, 2 * GS * HD:2 * GS * HD + HD]
                sk_t = tg[:, 2 * GS * HD + HD:TRIGW]

                k_tile = ps_s.tile([128, 2, CHUNK], F32, tag="s")
                k_ps = k_tile[:][:, 0, 0:HD]
                for t in range(NDK // 2):
                    nc.tensor.matmul(
                        k_ps, x8[:, 2 * t:2 * t + 2, :],
                        wk8_sb[:, 2 * t:2 * t + 2, :],
                        start=(t == 0), stop=(t == NDK // 2 - 1), perf_mode=DR,
                    )
                q_ps = ps_q.tile([128, GS * HD], F32, tag="q")
                for t in range(NDK // 2):
                    nc.tensor.matmul(
                        q_ps[:], x8[:, 2 * t:2 * t + 2, :],
                        wq8_sb[:, 2 * t:2 * t + 2, :],
                        start=(t == 0), stop=(t == NDK // 2 - 1),
                        perf_mode=DR,
                    )

                qsb = scrA.tile([128, GS * HD], BF16, tag="qsb")
                ksb = scrA.tile([128, HD], BF16, tag="ksb")
                nc.vector.tensor_copy(qsb[:], q_ps[:])
                nc.scalar.copy(ksb[:], k_ps)

                # rmsnorm stats (free-dim reduce per head)
                sq_full = scrA.tile([128, GS * HD], BF16, tag="sqf")
                sums = scrA.tile([128, 8], F32, tag="sums")
                rms = scrA.tile([128, 8], F32, tag="rms")
                recip = scrA.tile([128, 8], F32, tag="recip")
                nc.vector.tensor_mul(sq_full[:], qsb[:], qsb[:])
                nc.vector.reduce_sum(
                    sums[:, 0:GS],
                    sq_full[:].rearrange("p (h d) -> p h d", h=GS),
                    axis=mybir.AxisListType.X,
                )
                sq_k = scrA.tile([128, HD], BF16, tag="sqk")
                nc.vector.tensor_mul(sq_k[:], ksb[:], ksb[:])
                nc.vector.reduce_sum(
                    sums[:, GS:GS + 1], sq_k[:], axis=mybir.AxisListType.X
                )
                nc.scalar.activation(
                    rms[:, 0:GS], sums[:, 0:GS], AF.Sqrt,
                    scale=1.0 / HD, bias=epsq_sb[:],
                )
                nc.scalar.activation(
                    rms[:, GS:GS + 1], sums[:, GS:GS + 1], AF.Sqrt,
                    scale=1.0 / HD, bias=epsk_sb[:],
                )
                nc.vector.reciprocal(recip[:, 0:GS + 1], rms[:, 0:GS + 1])

                # normalize (q_scale/k_scale and k's S_D*SM are in the trig)
                qn = scrA.tile([128, GS * HD], BF16, tag="qn")
                for h in range(GS):
                    hs = slice(h * HD, (h + 1) * HD)
                    nc.vector.tensor_scalar_mul(
                        qn[:, hs], qsb[:, hs], recip[:, h:h + 1]
                    )
                kn = scrA.tile([128, HD], BF16, tag="kn")
                nc.vector.tensor_scalar_mul(
                    kn[:], ksb[:], recip[:, GS:GS + 1]
                )

                # rope: qr = qn*cos' + swap_halves(qn)*sin' (sign in sin')
                hh = HD // 2
                t1q = scrA.tile([128, GS * HD], BF16, tag="t1q")
                t2q = scrA.tile([128, GS * HD], BF16, tag="t2q")
                qr = scrA.tile([128, GS * HD], BF16, tag="qr")
                nc.vector.tensor_mul(t1q[:], qn[:], cq_t)
                qn3 = qn[:].rearrange("p (h d) -> p h d", h=GS)
                t23 = t2q[:].rearrange("p (h d) -> p h d", h=GS)
                sq3 = sq_t.rearrange("p (h d) -> p h d", h=GS)
                nc.vector.tensor_mul(
                    t23[:, :, 0:hh], qn3[:, :, hh:HD], sq3[:, :, 0:hh]
                )
                nc.vector.tensor_mul(
                    t23[:, :, hh:HD], qn3[:, :, 0:hh], sq3[:, :, hh:HD]
                )
                nc.vector.tensor_add(qr[:], t1q[:], t2q[:])

                t1k = scrA.tile([128, HD], BF16, tag="t1k")
                t2k = scrA.tile([128, HD], BF16, tag="t2k")
                kr = krblk_sb[:, lt % 4, :]
                nc.vector.tensor_mul(t1k[:], kn[:], ck_t)
                nc.vector.tensor_mul(t2k[:, 0:hh], kn[:, hh:HD], sk_t[:, 0:hh])
                nc.vector.tensor_mul(t2k[:, hh:HD], kn[:, 0:hh], sk_t[:, hh:HD])
                nc.vector.tensor_add(kr, t1k[:], t2k[:])

                # transpose q/k to [hd, l]
                tr = ps_tr.tile([128, 5, 128], BF16, tag="tr")
                for h in range(GS):
                    hs = slice(h * HD, (h + 1) * HD)
                    nc.tensor.transpose(tr[:, h, :], qr[:, hs], ident_sb[:])
                nc.tensor.transpose(tr[:, GS, :], kr, ident_sb[:])
                nc.vector.tensor_copy(qT_sb[:, :, ls], tr[:, 0:GS, :])
                nc.vector.tensor_copy(kT_sb[:, ls], tr[:, GS, :])

                # running column-sums of v (S-rows for the CSfullT inject)
                sp_tile = ps_acc.tile([128, 129], F32, tag="acc")
                sp = sp_tile[0:1, 0:128]
                nc.tensor.matmul(sp, ones_col[:], v_sb[:, lt, 0:HD])
                nc.vector.tensor_add(
                    srow_f[:, lt + 1, :], sp, srow_f[:, lt, :]
                )
                if lt + 1 < NLT:
                    nc.vector.tensor_copy(
                        srow_b[:, lt + 1, :], srow_f[:, lt + 1, :]
                    )

                # CSfullT/n for this tile: v^T @ M_lt + S-row x nrecip
                cs_tile = ps_acc.tile([128, 129], F32, tag="acc")
                cs_ps = cs_tile[:, 0:128]
                nc.tensor.matmul(
                    cs_ps, v_sb[:, lt, 0:HD], mi_sb[:, lt, :],
                    start=True, stop=(lt == 0),
                )
                if lt > 0:
                    nc.tensor.matmul(
                        cs_ps, srow_b[:, lt, :], nrow_sb[:, lt, :],
                        start=False, stop=True,
                    )
                nc.scalar.copy(csfT_sb[:, lt, :], cs_ps)

            def emit_Z(blk):
                """Z-prefix block burst: Z += K^T [V|1] over tiles
                4*blk..4*blk+3 (kr S_D*SM-scaled); z_sb[c] = prefix < 4c."""
                zp = ps_acc.tile([128, 129], F32, tag="acc")
                for j in range(4):
                    nc.tensor.matmul(
                        zp[:], krblk_sb[:, j, :], v_sb[:, 4 * blk + j, 0:129],
                        start=(j == 0), stop=(j == 3),
                    )
                if blk == 0:
                    nc.scalar.copy(z_sb[:, 1, 0:129], zp[:])
                else:
                    nc.vector.tensor_add(
                        z_sb[:, blk + 1, 0:129], zp[:],
                        z_sb[:, blk, 0:129],
                    )

            def emit_B_head(c, h, attn_my, ag_out, ag_sb, csn_my, cs_ag,
                            csag_sb):
                """attention for one (query-chunk, head): diag band in fp8
                DoubleRow + Z prefix apply; emits delta_a^T fp8."""
                cs = slice(c * CHUNK, (c + 1) * CHUNK)
                # diag scores: 2 pair-tiles
                d8s = []
                for pr in range(2):
                    s_ps = ps_s.tile([128, 2, CHUNK], F32, tag="s")
                    for j in range(2):
                        jt = 4 * c + 2 * pr + j
                        js = slice(jt * 128, (jt + 1) * 128)
                        nc.tensor.matmul(
                            s_ps[:, j, :], kT_sb[:, js], qT_sb[:, h, cs]
                        )
                    d8 = wTp.tile([128, 2, CHUNK], FP8, tag="w")
                    nc.scalar.copy(d8[:], s_ps[:])
                    nc.gpsimd.tensor_mul(
                        d8[:], d8[:], maskd_sb[:, 2 * pr:2 * pr + 2, :]
                    )
                    d8s.append(d8)
                da_nf = anp.tile([128, 4, 128], BF16, tag="anf")
                for qc in range(4):
                    qs = slice(qc * 128, (qc + 1) * 128)
                    acc = ps_acc.tile([128, 129], F32, tag="acc")
                    nprs = 1 if qc < 2 else 2
                    nmm = nprs + (1 if c > 0 else 0)
                    for pr in range(nprs):
                        jt = 4 * c + 2 * pr
                        nc.tensor.matmul(
                            acc[:], d8s[pr][:, :, qs],
                            v8_sb[:, jt:jt + 2, 0:129],
                            start=(pr == 0), stop=(pr == nmm - 1),
                            perf_mode=DR,
                        )
                    if c > 0:
                        # Z prefix apply: off-diag (s*mask)@v collapsed
                        qg = slice(c * CHUNK + qc * 128,
                                   c * CHUNK + (qc + 1) * 128)
                        nc.tensor.matmul(
                            acc[:], qT_sb[:, h, qg],
                            z_sb[:, c, 0:129],
                            start=False, stop=True,
                        )
                    # rs' = accd_col/S_A + n*S_D/S_A ; rec = S_A/(S_D*rs)
                    rsv = scrB.tile([128, 1], F32, tag="rsv")
                    rec = scrB.tile([128, 1], F32, tag="rec")
                    nc.vector.scalar_tensor_tensor(
                        rsv[:], acc[:, 128:129], 1.0 / S_A,
                        nvec_sb[:, 4 * c + qc:4 * c + qc + 1],
                        op0=ALU.mult, op1=ALU.add,
                    )
                    nc.vector.reciprocal(rec[:], rsv[:])
                    nc.vector.tensor_scalar_mul(
                        da_nf[:, qc, :], acc[:, 0:128], rec[:]
                    )
                tr = ps_tr.tile([128, 5, 128], BF16, tag="tr")
                for qc in range(4):
                    nc.tensor.transpose(
                        tr[:, qc, :], da_nf[:][:, qc, :], ident_sb[:]
                    )
                aT = aTp.tile([128, 4, 128], FP8, tag="aT")
                nc.scalar.copy(aT[:], tr[:, 0:4, :])
                nc.gpsimd.dma_start(
                    attn_my[h * HD:(h + 1) * HD, :],
                    aT[:].rearrange("p q d -> p (q d)"),
                )
                if h == 0:
                    # this core's CSnT slice for the chunk (head-indep)
                    nc.gpsimd.dma_start(
                        csn_my[:, :],
                        csfT_sb[:, 4 * c:4 * c + 4, :]
                        .rearrange("p t n -> p (t n)"),
                    )
                if sim_mode:
                    # per-head AllGather stand-in (same DMA volume as the
                    # real 4-way gather of this head's quarter)
                    ag_q = ag_out[h * GS * HD:(h + 1) * GS * HD, :]
                    nc.sync.dma_start(
                        ag_q.rearrange("(r p) n -> p r n", p=128),
                        aT[:].rearrange("p q d -> p (q d)")
                        .rearrange("p (o n) -> p o n", o=1)
                        .to_broadcast([128, 4, CHUNK]),
                    )
                    ag_v = ag_out[:].rearrange("(t p) n -> p t n", p=128)
                    nc.sync.dma_start(
                        ag_sb[:, 4 * h:4 * h + 4, :],
                        ag_v[:, 4 * h:4 * h + 4, :],
                    )
                    if h == 0:
                        csv = cs_ag[:].rearrange("(t p) n -> p t n", p=128)
                        nc.sync.dma_start(
                            csv[:],
                            csfT_sb[:, 4 * c:4 * c + 4, :]
                            .rearrange("p t n -> p (t n)")
                            .rearrange("p (o n) -> p o n", o=1)
                            .to_broadcast([128, 4, CHUNK]),
                        )
                        nc.sync.dma_start(csag_sb[:], csv[:])

            ag_sbs = {}
            csag_sbs = {}
            ag_ccs = {}
            cs_ccs = {}

            def emit_AG(c, attn_my, ag_out, csn_my, cs_ag):
                if sim_mode:
                    return
                cc = nc.gpsimd.collective_compute(
                    "AllGather",
                    ALU.bypass,
                    ins=[attn_my.opt()],
                    outs=[ag_out.opt()],
                    replica_groups=REPLICA_GROUPS,
                )
                ag_ccs[c] = cc
                cc2 = nc.gpsimd.collective_compute(
                    "AllGather",
                    ALU.bypass,
                    ins=[csn_my.opt()],
                    outs=[cs_ag.opt()],
                    replica_groups=REPLICA_GROUPS,
                )
                cs_ccs[c] = cc2

            def emit_real_loads(c):
                ag_v = ag_outs[c][:].rearrange("(t p) n -> p t n", p=128)
                ag_sb = ag_sbs[c]
                for g2 in range(4):
                    ld = nc.sync.dma_start(
                        ag_sb[:, 4 * g2:4 * g2 + 4, :],
                        ag_v[:, 4 * g2:4 * g2 + 4, :],
                    )
                    tile.add_dep_helper(
                        ld.ins, ag_ccs[c].ins, reason="ag load after collective"
                    )
                csv = cs_ags[c][:].rearrange("(t p) n -> p t n", p=128)
                ld2 = nc.sync.dma_start(csag_sbs[c][:], csv[:])
                tile.add_dep_helper(
                    ld2.ins, cs_ccs[c].ins, reason="cs load after collective"
                )

            def emit_C_tile(c, it, alt_bank=False):
                ag_sb = ag_sbs[c]
                csag_sb = csag_sbs[c]
                its = slice(it * 128, (it + 1) * 128)
                if alt_bank:
                    o_tile = ps_s.tile([128, 2, CHUNK], F32, tag="s")
                    o_ps = o_tile[:][:, 0, :]
                else:
                    o_tile = ps_q.tile([128, GS * HD], F32, tag="q")
                    o_ps = o_tile[:]
                for t in range(0, H, 2):
                    nc.tensor.matmul(
                        o_ps, ag_sb[:, t:t + 2, its], wo8_sb[:, t:t + 2, :],
                        start=(t == 0), stop=False, perf_mode=DR,
                    )
                for g2 in range(G):
                    nc.tensor.matmul(
                        o_ps, csag_sb[:, g2, its], wosum_sb[:, g2, :],
                        start=False, stop=(g2 == G - 1),
                    )
                o_sb = outpool.tile([128, CHUNK], F32, tag="o_sb")
                nc.scalar.activation(o_sb[:], o_ps, AF.Copy, scale=OUT_DESCALE)
                nc.scalar.dma_start(
                    out[c * CHUNK + it * 128:c * CHUNK + (it + 1) * 128, :],
                    o_sb[:],
                )

            # ---------------- staircase schedule ----------------------------
            ag_outs = {}
            cs_ags = {}
            z_ps_tiles = {}
            pre0 = emit_A_dma(0)
            nc.sync.dma_start(wk8_sb[:], wk8[:])
            nc.sync.dma_start(wv_sb[:], wv[:])
            nc.sync.dma_start(wq8_sb[:, 0:NDK // 2, :], wq8[:, 0:NDK // 2, :])
            nc.sync.dma_start(wq8_sb[:, NDK // 2:, :], wq8[:, NDK // 2:, :])
            nc.sync.dma_start(mi_sb[:], mi[:])
            nc.sync.dma_start(nrow_sb[:], nrow[:])
            nc.sync.dma_start(nvec_sb[:], nvec[:])
            emit_VT(0)
            emit_A(0, pre=pre0)
            nc.sync.dma_start(maskd_sb[:], maskd_v)
            for lt in range(1, 4):
                emit_A(lt)
            emit_Z(0)
            emit_VT(1)
            for lt in range(4, 8):
                emit_A(lt)
            emit_Z(1)
            emit_VT(2)
            for c in range(NCH):
                if c == 1:
                    with tc.tile_wait_until(0.06):
                        nc.gpsimd.dma_start(wo8_sb[:], wo8[:])
                        nc.gpsimd.dma_start(wosum_sb[:], wosum[:])
                attn_my = ccpool.tile([GS * HD, CHUNK], FP8, tag="attn_my")
                ag_out = ccagpool.tile([H * HD, CHUNK], FP8, tag="ag_out")
                csn_my = cspool.tile([HD, CHUNK], BF16, tag="csn_my")
                cs_ag = csagpool.tile([G * HD, CHUNK], BF16, tag="cs_ag")
                ag_sb = aginpool.tile([128, H, CHUNK], FP8, tag="ag")
                csag_sb = csinpool.tile([128, G, CHUNK], BF16, tag="csag")
                ag_sbs[c] = ag_sb
                csag_sbs[c] = csag_sb
                for h in range(GS):
                    emit_B_head(c, h, attn_my, ag_out, ag_sb, csn_my,
                                cs_ag, csag_sb)
                    lt = 4 * c + 8 + h
                    if lt < NLT:
                        if lt == 12:
                            emit_VT(3)
                        emit_A(lt)
                        if lt % 4 == 3 and lt // 4 < NCH - 1:
                            emit_Z(lt // 4)
                    if c >= 2:
                        emit_C_tile(c - 2, h)
                ag_outs[c] = ag_out
                cs_ags[c] = cs_ag
                emit_AG(c, attn_my, ag_out, csn_my, cs_ag)
                if not sim_mode and c >= 1:
                    emit_real_loads(c - 1)
            for it in range(4):
                emit_C_tile(2, it, alt_bank=(it % 2 == 1))
            if not sim_mode:
                emit_real_loads(3)
            for it in range(4):
                emit_C_tile(3, it, alt_bank=(it % 2 == 1))
    nc.compile()
    return nc


def _get_nc():
    if "nc" not in _CACHE:
        _CACHE["nc"] = _build_bass()
    return _CACHE["nc"]


def kernel(x, Wq, Wk, Wv, Wo, q_scale, k_scale, cos, sin, mask):
    global LAST_RESULT
    nc = _get_nc()

    f32 = np.float32
    bf16 = ml_dtypes.bfloat16
    fp8 = ml_dtypes.float8_e4m3fn
    x = np.asarray(x, f32)
    cos = np.asarray(cos, f32)
    sin = np.asarray(sin, f32)
    q_scale = np.asarray(q_scale, f32)
    k_scale = np.asarray(k_scale, f32)

    sgn = np.concatenate([-np.ones(HD // 2, f32), np.ones(HD // 2, f32)])
    qs_swap = np.concatenate([q_scale[HD // 2:], q_scale[:HD // 2]])
    ks_swap = np.concatenate([k_scale[HD // 2:], k_scale[:HD // 2]])
    cosq = np.tile(cos * q_scale[None, :], (1, GS))  # [L, GS*HD]
    sinq = np.tile(sin * (sgn * qs_swap)[None, :], (1, GS))
    ksc = S_D * SM_SCALE  # scores arrive in psum pre-scaled by S_D*SM
    cosk = cos * (k_scale * ksc)[None, :]  # [L, HD]
    sink = sin * (sgn * ks_swap * ksc)[None, :]
    trig_full = np.concatenate([cosq, sinq, cosk, sink], axis=1)  # [L, 1280]
    trig_t = np.ascontiguousarray(
        trig_full.reshape(NLT, 128, -1).astype(bf16))  # [lt, p, 1280]

    # diagonal-band mask, key-major: 1.0 where key j' may attend query i'
    maskd = np.ascontiguousarray((~mask[:CHUNK, :CHUNK]).T.astype(fp8))

    # causal-mean static matrices: M_i[k, l] = (k<=l)/n_l, n_l = 128*i+l+1
    ltri = np.triu(np.ones((128, 128), f32))
    mi_t = np.empty((128, NLT, 128), f32)
    nrow_t = np.empty((1, NLT, 128), f32)
    nvec_t = np.empty((128, NLT), f32)
    for i in range(NLT):
        n_l = 128.0 * i + np.arange(128, dtype=f32) + 1.0
        mi_t[:, i, :] = ltri / n_l[None, :]
        nrow_t[0, i, :] = 1.0 / n_l
        nvec_t[:, i] = n_l * (S_D / S_A)

    # x tiled [lt, p, dk, l] fp8 (q/k) and v-layout [c, p, dk, 512] bf16
    xt8s, xtvs = [], []
    for b in range(B):
        xr = np.asarray(x[b], f32).reshape(NLT, 128, NDK, 128)  # [lt,l,dk,p]
        xr8 = np.ascontiguousarray(xr.transpose(0, 3, 2, 1))  # [lt,p,dk,l]
        xt8s.append(xr8.astype(fp8).reshape(NLT, 128, NDK * 128))
        xv = np.asarray(x[b], f32).reshape(NCH, 512, NDK, 128)
        xv = np.ascontiguousarray(xv.transpose(0, 3, 2, 1))  # [c,p,dk,512]
        xtvs.append(xv.astype(bf16).reshape(NCH, 128, NDK * 512))

    def pack_pdn(w):  # [D, N] -> [p, dk, N]
        n = w.shape[1]
        return np.ascontiguousarray(
            w.reshape(NDK, 128, n).transpose(1, 0, 2))

    in_maps = []
    for core in range(NCORES):
        b, g = divmod(core, G)
        hs = slice(g * GS * HD, (g + 1) * GS * HD)
        gs = slice(g * HD, (g + 1) * HD)
        wo_my = np.asarray(Wo[:, g * CHUNK:(g + 1) * CHUNK], f32)  # [2048,512]
        # per-group sums of Wo head-blocks, pre-scaled for the psum descale
        wosum_my = (wo_my.reshape(G, GS, HD, CHUNK).sum(axis=1)
                    * (S_A * WQ_SCALE))  # [G, HD, CHUNK]
        wosum_my = np.ascontiguousarray(
            wosum_my.transpose(1, 0, 2)).astype(bf16)  # [p, G, CHUNK]
        in_maps.append({
            "xt8": xt8s[b],
            "xtv": xtvs[b],
            "wq8": pack_pdn((np.asarray(Wq[:, hs], f32) * WQ_SCALE)).astype(fp8),
            "wk8": pack_pdn((np.asarray(Wk[:, gs], f32) * WQ_SCALE)).astype(fp8),
            "wv": pack_pdn(np.asarray(Wv[:, gs], f32)).astype(bf16),
            "wo8": pack_pdn(wo_my * WQ_SCALE).astype(fp8),
            "wosum": wosum_my,
            "trig": trig_t,
            "maskd8": maskd,
            "mi": mi_t.astype(bf16),
            "nrow": nrow_t.astype(bf16),
            "nvec": nvec_t,
            "ident": np.eye(128, dtype=bf16),
        })

    res = run_bass_kernel_spmd(nc, in_maps, list(range(NCORES)))
    LAST_RESULT = res

    out = np.empty((B, L, D), f32)
    for core in range(NCORES):
        b, g = divmod(core, G)
        out[b, :, g * CHUNK:(g + 1) * CHUNK] = res.results[core]["out"]
    return out
